# revision 16
# baseline (speedup 1.0000x reference)
"""Trainium2 Bass kernel for nn_AttentionCellEncoder.

Contract: kernel(**inputs) takes FULL unsharded inputs (as produced by
setup_inputs) and returns the FULL [2048, 256] float32 output. Internally
shards cells across 8 NeuronCores (data-parallel over the cell dimension,
chunk_features table replicated), runs a Bass/Tile kernel via
run_bass_kernel_spmd, and reassembles the output.

Self-contained: all shapes/sharding hardcoded.

Numerics: all large matmuls run in bf16 (operands rounded to bf16, fp32
PSUM accumulation); the final [512->256] projection stays fp32. The
masked mean-pool divides by cell_len on the host (pool weights are an
exact 0/1 bf16 mask). Validated ~2.8e-3 max rel error vs the fp32
reference (threshold 2e-2).
"""

import numpy as np
import ml_dtypes

import concourse.bass as bass
import concourse.mybir as mybir
import concourse.tile as tile
from concourse import bacc
from concourse.bass_utils import run_bass_kernel_spmd
from concourse.masks import make_identity

FP = mybir.dt.float32
BF = mybir.dt.bfloat16
F8 = mybir.dt.float8e4
P = 128

# Problem dims
NUM_HEADS = 8
NUM_CHUNKS, INPUT_DIM = 50000, 768   # D = 768
HIDDEN_DIM, OUTPUT_DIM = 512, 256    # H = 512
NUM_CELLS, MAX_LEN = 2048, 64        # C, L
HEAD_DIM = HIDDEN_DIM // NUM_HEADS   # 64
N_CORES = 8
CELLS_PER_CORE = NUM_CELLS // N_CORES          # 256
TILES_PER_CORE = CELLS_PER_CORE // 2           # 128 tiles of 2 cells / 128 tokens
TILES_PER_BLOCK = 4                            # 512 tokens per block
BLOCKS = TILES_PER_CORE // TILES_PER_BLOCK     # 32
DCH = INPUT_DIM // P                           # 6 d-chunks
HCH = HIDDEN_DIM // P                          # 4 h-chunks
TOK_BLK = TILES_PER_BLOCK * P                  # 512
CELL_GROUPS = CELLS_PER_CORE // P              # 2 output groups of 128 cells


# Debug/bisection switches (must match between build_kernel and preprocess):
#   use_swap:   baseline-style swapped qT/kT copies + diagonal-quad scores.
#               MUST stay True: matmuls whose input partition half differs
#               from the output partition half (off-diagonal PE tile_position)
#               produce wrong results on TRN2 hardware (CoreSim accepts them).
#   fp32_gather: keep table/x/transposes in fp32, convert to bf16 at copy-out
CFG = {"use_swap": True, "fp32_gather": False, "fp8_qk": True}

# fp8 scaling: weights/x are pre-scaled into e4m3 range; the q/k activation
# descales via its scale operand. Scores and everything downstream unchanged.
SX = 16.0      # x scale for the fp8 copy (applied on the ACT conversion)
SWQ = 512.0    # wq_eff scale (entries ~1e-3 -> ~0.5)
SWK = 64.0     # wk_eff scale (entries ~9e-3 -> ~0.5)


def build_kernel(flags, repeat: int = 1, att_bufs: int = 2,
                 blk_bufs: int = 2, x_bufs: int = 5,
                 xp_bufs: int = 2, acc_bufs: int = 2):
    """Trace and compile the per-core SPMD kernel. Returns the Bass object.
    flags: (with_v_bias, with_qk_bias) or a bare bool for with_v_bias."""
    if isinstance(flags, tuple):
        with_v_bias, with_qk_bias = flags
    else:
        with_v_bias, with_qk_bias = flags, False
    use_swap = CFG["use_swap"]
    fp32_gather = CFG["fp32_gather"]
    fp8_qk = CFG["fp8_qk"]
    QKDT = F8 if fp8_qk else BF
    nc = bacc.Bacc(None)

    GDT = FP if fp32_gather else BF      # gather/transpose-path dtype
    table = nc.dram_tensor("table", [NUM_CHUNKS, INPUT_DIM], GDT, kind="ExternalInput")
    wq_t = nc.dram_tensor("wq_t", [INPUT_DIM, HIDDEN_DIM], QKDT, kind="ExternalInput")
    wk_t = nc.dram_tensor("wk_t", [INPUT_DIM, HIDDEN_DIM], QKDT, kind="ExternalInput")
    wv_t = nc.dram_tensor("wv_t", [INPUT_DIM, HIDDEN_DIM], BF, kind="ExternalInput")
    wf_t = nc.dram_tensor("wf_t", [HIDDEN_DIM, OUTPUT_DIM], FP, kind="ExternalInput")
    bq_c = nc.dram_tensor("bq_c", [P, HCH], FP, kind="ExternalInput")
    bk_c = nc.dram_tensor("bk_c", [P, HCH], FP, kind="ExternalInput")
    bv_r = nc.dram_tensor("bv_r", [1, HIDDEN_DIM], BF, kind="ExternalInput")
    idx = nc.dram_tensor("idx", [BLOCKS * P, TILES_PER_BLOCK], mybir.dt.int32,
                         kind="ExternalInput")
    maskb = nc.dram_tensor("maskb", [BLOCKS * P, TILES_PER_BLOCK], FP,
                           kind="ExternalInput")
    u2 = nc.dram_tensor("u2", [BLOCKS * P, 2 * TILES_PER_BLOCK], BF,
                        kind="ExternalInput")
    out = nc.dram_tensor("out", [CELLS_PER_CORE, OUTPUT_DIM], FP,
                         kind="ExternalOutput")

    with tile.TileContext(nc) as tc:
        with (
            tc.tile_pool(name="const", bufs=1) as cpool,
            tc.tile_pool(name="xp", bufs=x_bufs) as xpool,
            tc.tile_pool(name="blk", bufs=blk_bufs) as bpool,
            tc.tile_pool(name="sm", bufs=3) as spool,
            tc.tile_pool(name="op", bufs=2) as opool,
            tc.tile_pool(name="ps", bufs=2, space="PSUM") as pspool,
        ):
            ident = cpool.tile([P, P], GDT)
            make_identity(nc, ident[:])
            ones = cpool.tile([P, 1], BF)
            nc.gpsimd.memset(ones[:], 1.0)

            wq_sb = cpool.tile([P, DCH * HIDDEN_DIM], QKDT)
            wk_sb = cpool.tile([P, DCH * HIDDEN_DIM], QKDT)
            wv_sb = cpool.tile([P, DCH * HIDDEN_DIM], BF)
            for wsb, wt in ((wq_sb, wq_t), (wk_sb, wk_t), (wv_sb, wv_t)):
                nc.sync.dma_start(
                    out=wsb[:].rearrange("p (j n) -> p j n", j=DCH),
                    in_=wt[:].rearrange("(j p) n -> p j n", j=DCH))
            wf_sb = cpool.tile([P, HCH * OUTPUT_DIM], FP)
            nc.sync.dma_start(
                out=wf_sb[:].rearrange("p (c n) -> p c n", c=HCH),
                in_=wf_t[:].rearrange("(c p) n -> p c n", c=HCH))
            bq_sb = cpool.tile([P, HCH], FP)
            bk_sb = cpool.tile([P, HCH], FP)
            nc.sync.dma_start(out=bq_sb[:], in_=bq_c[:, :])
            nc.sync.dma_start(out=bk_sb[:], in_=bk_c[:, :])
            if with_v_bias:
                ones1 = cpool.tile([1, P], BF)
                nc.gpsimd.memset(ones1[:], 1.0)
                bv_sb = cpool.tile([1, HIDDEN_DIM], BF)
                nc.sync.dma_start(out=bv_sb[:], in_=bv_r[:, :])

            for rep in range(repeat):
                # Software-pipelined: per iteration, emit in this order:
                #   loads(b)+gathers(b) -> scores+exp(b-1) -> transposes(b)
                #   -> qk(b)+swaps -> v(b) -> ctx/normalize(b-1) -> pool(b-1)
                # so the PE always has dense matmul work queued while the
                # previous block's exp/normalize chain runs on ACT/DVE, and
                # gathers land while scores(b-1) occupy the PE.
                # pooled accumulates in SBUF (poolSB) via one DVE add per block.
                poolSB = [None] * CELL_GROUPS
                prev = None
                for b in range(BLOCKS + 1):
                    cur = None
                    if b < BLOCKS:
                        # ---- block-level small loads + all 4 gathers ----
                        idx_sb = spool.tile([P, TILES_PER_BLOCK], mybir.dt.int32,
                                            tag="idx")
                        nc.sync.dma_start(out=idx_sb[:],
                                          in_=idx[b * P:(b + 1) * P, :])
                        mk = spool.tile([P, TILES_PER_BLOCK], FP, tag="mk")
                        nc.sync.dma_start(out=mk[:],
                                          in_=maskb[b * P:(b + 1) * P, :])
                        u2_sb = spool.tile([P, 2 * TILES_PER_BLOCK], BF, tag="u2")
                        nc.sync.dma_start(out=u2_sb[:],
                                          in_=u2[b * P:(b + 1) * P, :])
                        xs = []
                        for t in range(TILES_PER_BLOCK):
                            x = xpool.tile([P, INPUT_DIM], GDT, tag="x")
                            nc.gpsimd.indirect_dma_start(
                                out=x[:], out_offset=None, in_=table[:],
                                in_offset=bass.IndirectOffsetOnAxis(
                                    ap=idx_sb[:, t:t + 1], axis=0),
                            )
                            xs.append(x)

                    if prev is not None:
                        # ---- attention stage 1 (b-1): scores + exp ----
                        (pb, mkp, u2p, qTp, kTp, qswp, kswp, vp) = prev
                        g = pb // (BLOCKS // CELL_GROUPS)
                        es = []
                        for t in range(TILES_PER_BLOCK):
                            # scores^T: [2c x 64 m, 8h x 64 l]; inputs and the
                            # output must sit on the same partition half
                            # (diagonal PE quads; off-diagonal breaks on HW) ->
                            # swapped copies supply the off-parity heads.
                            sc = pspool.tile([P, HIDDEN_DIM], FP, tag="sc",
                                             bufs=att_bufs)
                            for h in range(NUM_HEADS):
                                hc = h // 2
                                for c in range(2):   # c inner: T0/T10 overlap
                                    fw = slice(hc * TOK_BLK + t * P + c * 64,
                                               hc * TOK_BLK + t * P + c * 64 + 64)
                                    pr = slice(c * 64, c * 64 + 64)
                                    kk, qq = ((kTp, qTp) if h % 2 == c
                                              else (kswp, qswp))
                                    nc.tensor.matmul(
                                        out=sc[pr, h * 64:h * 64 + 64],
                                        lhsT=kk[pr, fw], rhs=qq[pr, fw],
                                        start=True, stop=True,
                                    )
                            e = spool.tile([P, HIDDEN_DIM], BF, tag="e", bufs=4)
                            nc.scalar.activation(
                                out=e[:], in_=sc[:],
                                func=mybir.ActivationFunctionType.Exp,
                                bias=mkp[:, t:t + 1])
                            es.append(e)

                    if b < BLOCKS:
                        # ---- transposes + xT/xT8 copies ----
                        xT = bpool.tile([P, DCH * TOK_BLK], BF, tag="xT")
                        if fp8_qk:
                            xT8 = bpool.tile([P, DCH * TOK_BLK], F8, tag="xT8")
                        for t in range(TILES_PER_BLOCK):
                            p6 = pspool.tile([P, DCH * P], GDT, tag="xp",
                                             bufs=xp_bufs)
                            for j in range(DCH):
                                nc.tensor.transpose(out=p6[:, j * P:(j + 1) * P],
                                                    in_=xs[t][:, j * P:(j + 1) * P],
                                                    identity=ident[:])
                            nc.vector.tensor_copy(
                                out=xT[:].rearrange("p (j n) -> p j n", j=DCH)
                                    [:, :, t * P:(t + 1) * P],
                                in_=p6[:].rearrange("p (j n) -> p j n", j=DCH),
                            )
                            if fp8_qk:
                                # GPSIMD cannot read PSUM: convert from the
                                # SBUF bf16 copy instead of from p6
                                nc.gpsimd.tensor_copy(
                                    out=xT8[:].rearrange("p (j n) -> p j n", j=DCH)
                                        [:, :, t * P:(t + 1) * P],
                                    in_=xT[:].rearrange("p (j n) -> p j n", j=DCH)
                                        [:, :, t * P:(t + 1) * P],
                                )

                        # ---- qT, kT: weight-stationary, N=512 tokens ----
                        # layout: [128 part = 2 heads x 64 d, HCH chunks x 512 tok]
                        qT = bpool.tile([P, HCH * TOK_BLK], BF, tag="qT")
                        kT = bpool.tile([P, HCH * TOK_BLK], BF, tag="kT")
                        qT_sw = kT_sw = None
                        if use_swap:
                            qT_sw = bpool.tile([P, HCH * TOK_BLK], BF, tag="qTsw")
                            kT_sw = bpool.tile([P, HCH * TOK_BLK], BF, tag="kTsw")
                        for (wsb, bsb, dst, dsc) in (
                                (wq_sb, bq_sb, qT, 1.0 / SWQ),
                                (wk_sb, bk_sb, kT, 1.0 / SWK)):
                            for hc in range(HCH):
                                acc = pspool.tile([P, TOK_BLK], FP, tag="acc",
                                                  bufs=acc_bufs)
                                if fp8_qk:
                                    w3 = wsb[:].rearrange("p (j h) -> p j h", j=DCH)
                                    x3 = xT8[:].rearrange("p (j n) -> p j n", j=DCH)
                                    for jj in range(DCH // 2):
                                        nc.tensor.matmul(
                                            out=acc[:],
                                            lhsT=w3[:, 2 * jj:2 * jj + 2,
                                                    hc * P:(hc + 1) * P],
                                            rhs=x3[:, 2 * jj:2 * jj + 2, :],
                                            start=(jj == 0),
                                            stop=(jj == DCH // 2 - 1),
                                            perf_mode=mybir.MatmulPerfMode.DoubleRow,
                                        )
                                else:
                                    for j in range(DCH):
                                        nc.tensor.matmul(
                                            out=acc[:],
                                            lhsT=wsb[:, j * HIDDEN_DIM + hc * P:
                                                     j * HIDDEN_DIM + (hc + 1) * P],
                                            rhs=xT[:, j * TOK_BLK:(j + 1) * TOK_BLK],
                                            start=(j == 0), stop=(j == DCH - 1),
                                        )
                                dslc = dst[:, hc * TOK_BLK:(hc + 1) * TOK_BLK]
                                if with_qk_bias:
                                    # bias nonzero: ACT applies scale then bias
                                    nc.scalar.activation(
                                        out=dslc, in_=acc[:],
                                        func=mybir.ActivationFunctionType.Identity,
                                        bias=bsb[:, hc:hc + 1],
                                        scale=(dsc if fp8_qk else 1.0))
                                elif dst is qT:
                                    # zero bias: pure descale; q on ACT, k on
                                    # DVE to halve the acc ping-pong latency
                                    nc.scalar.activation(
                                        out=dslc, in_=acc[:],
                                        func=mybir.ActivationFunctionType.Copy,
                                        scale=(dsc if fp8_qk else 1.0))
                                elif fp8_qk:
                                    nc.vector.tensor_scalar_mul(
                                        out=dslc, in0=acc[:], scalar1=dsc)
                                else:
                                    nc.vector.tensor_copy(out=dslc, in_=acc[:])
                            if use_swap:
                                dsw = qT_sw if dst is qT else kT_sw
                                nc.sync.dma_start(out=dsw[0:64, :], in_=dst[64:P, :])
                                nc.sync.dma_start(out=dsw[64:P, :], in_=dst[0:64, :])

                        # ---- v: x-stationary per tile, [128 tok, 512 h] ----
                        v = bpool.tile([P, TILES_PER_BLOCK * HIDDEN_DIM], BF,
                                       tag="v")
                        for t in range(TILES_PER_BLOCK):
                            acc = pspool.tile([P, HIDDEN_DIM], FP, tag="acc")
                            nmm = DCH + (1 if with_v_bias else 0)
                            for j in range(DCH):
                                nc.tensor.matmul(
                                    out=acc[:],
                                    lhsT=xT[:, j * TOK_BLK + t * P:
                                            j * TOK_BLK + (t + 1) * P],
                                    rhs=wv_sb[:, j * HIDDEN_DIM:(j + 1) * HIDDEN_DIM],
                                    start=(j == 0), stop=(j == nmm - 1),
                                )
                            if with_v_bias:
                                nc.tensor.matmul(out=acc[:], lhsT=ones1[0:1, :],
                                                 rhs=bv_sb[0:1, :],
                                                 start=False, stop=True)
                            nc.scalar.activation(
                                out=v[:, t * HIDDEN_DIM:(t + 1) * HIDDEN_DIM],
                                in_=acc[:],
                                func=mybir.ActivationFunctionType.Copy)
                        cur = (b, mk, u2_sb, qT, kT, qT_sw, kT_sw, v)

                    if prev is not None:
                        # ---- attention stage 2 (b-1): ctx + normalize ----
                        cns = []
                        for t in range(TILES_PER_BLOCK):
                            e = es[t]
                            ctx = pspool.tile([P, HIDDEN_DIM], FP, tag="ctx",
                                              bufs=att_bufs)
                            sden = pspool.tile([P, NUM_HEADS], FP, tag="ctx",
                                               bufs=att_bufs)
                            for h in range(NUM_HEADS):
                                for c in range(2):
                                    el = e[c * 64:c * 64 + 64, h * 64:h * 64 + 64]
                                    nc.tensor.matmul(
                                        out=ctx[c * 64:c * 64 + 64,
                                                h * 64:h * 64 + 64],
                                        lhsT=el,
                                        rhs=vp[c * 64:c * 64 + 64,
                                               t * HIDDEN_DIM + h * 64:
                                               t * HIDDEN_DIM + h * 64 + 64],
                                        start=True, stop=True,
                                    )
                                    nc.tensor.matmul(
                                        out=sden[c * 64:c * 64 + 64, h:h + 1],
                                        lhsT=el, rhs=ones[c * 64:c * 64 + 64, 0:1],
                                        start=True, stop=True,
                                    )
                            r = spool.tile([P, NUM_HEADS], FP, tag="r", bufs=4)
                            nc.vector.reciprocal(out=r[:], in_=sden[:])
                            cn = spool.tile([P, HIDDEN_DIM], BF, tag="cn", bufs=4)
                            nc.vector.tensor_tensor(
                                out=cn[:].rearrange("p (h d) -> p h d",
                                                    h=NUM_HEADS),
                                in0=ctx[:].rearrange("p (h d) -> p h d",
                                                     h=NUM_HEADS),
                                in1=r[:, :, None].to_broadcast(
                                    [P, NUM_HEADS, HEAD_DIM]),
                                op=mybir.AluOpType.mult,
                            )
                            cns.append(cn)
                        # ---- stage 3: pooled columns -> one SBUF accumulate
                        # pool32[:, hc*8 + 2t + c] = sum_l u2[l,c]*cn_t[l, hc*128+.]
                        pool32 = pspool.tile([P, HCH * 2 * TILES_PER_BLOCK], FP,
                                             tag="sc", bufs=att_bufs)
                        for t in range(TILES_PER_BLOCK):
                            for hc in range(HCH):
                                nc.tensor.matmul(
                                    out=pool32[:, hc * 8 + 2 * t:hc * 8 + 2 * t + 2],
                                    lhsT=cns[t][:, hc * P:(hc + 1) * P],
                                    rhs=u2p[:, 2 * t:2 * t + 2],
                                    start=True, stop=True,
                                )
                        if poolSB[g] is None:
                            poolSB[g] = opool.tile([P, HIDDEN_DIM], FP,
                                                   tag=f"poolSB{g}", bufs=1,
                                                   name=f"poolSB{g}")
                        # each block owns 8 disjoint columns per hc-chunk
                        boff = (pb % (BLOCKS // CELL_GROUPS)) * (2 * TILES_PER_BLOCK)
                        nc.vector.tensor_copy(
                            out=poolSB[g][:].rearrange("p (hc n) -> p hc n",
                                                       hc=HCH)[:, :, boff:boff + 8],
                            in_=pool32[:].rearrange("p (hc n) -> p hc n", hc=HCH))
                    prev = cur

                # ---- final projection per group of 128 cells (fp32) ----
                for g in range(CELL_GROUPS):
                    acc = pspool.tile([P, OUTPUT_DIM], FP, tag="acc")
                    for c in range(HCH):
                        nc.tensor.matmul(
                            out=acc[:], lhsT=poolSB[g][:, c * P:(c + 1) * P],
                            rhs=wf_sb[:, c * OUTPUT_DIM:(c + 1) * OUTPUT_DIM],
                            start=(c == 0), stop=(c == HCH - 1),
                        )
                    osb = opool.tile([P, OUTPUT_DIM], FP, tag="osb")
                    nc.scalar.activation(out=osb[:], in_=acc[:],
                                         func=mybir.ActivationFunctionType.Copy)
                    nc.sync.dma_start(out=out[g * P:(g + 1) * P, :], in_=osb[:])

    nc.compile()
    return nc


def preprocess(chunk_features, Wq, bq, Wk, bk, Wv, bv, W_in, b_in, Wo, bo,
               Wout, bout, cell_idx, cell_len):
    """Host-side weight folding + per-core input maps. Returns (in_maps, b_final,
    inv_len, with_v_bias)."""
    f32 = np.float32
    bf16 = ml_dtypes.bfloat16
    gdt = f32 if CFG["fp32_gather"] else bf16
    tscale = f32(SX) if CFG["fp8_qk"] else f32(1.0)
    cf = np.ascontiguousarray((np.asarray(chunk_features, f32) * tscale).astype(gdt))
    Wq, Wk, Wv = (np.asarray(w, f32) for w in (Wq, Wk, Wv))
    bq, bk, bv = (np.asarray(x, f32) for x in (bq, bk, bv))
    W_in = np.asarray(W_in, f32)
    b_in = np.asarray(b_in, f32)
    Wo, bo = np.asarray(Wo, f32), np.asarray(bo, f32)
    Wout, bout = np.asarray(Wout, f32), np.asarray(bout, f32)

    Wiq, Wik, Wiv = np.split(W_in, 3, axis=0)
    biq, bik, biv = np.split(b_in, 3)
    scale = f32(1.0 / np.sqrt(HEAD_DIM))
    wq_eff = (Wiq @ Wq) * scale          # [512, 768]
    wk_eff = Wik @ Wk
    wv_eff = Wiv @ Wv
    bq_eff = (Wiq @ bq + biq) * scale    # [512]
    bk_eff = Wik @ bk + bik
    bv_eff = Wiv @ bv + biv
    wfin = Wout @ Wo                     # [256, 512]
    b_final = bo @ Wout.T + bout         # [256]

    if CFG["fp8_qk"]:
        f8 = ml_dtypes.float8_e4m3
        wq_t = np.ascontiguousarray((wq_eff.T * (SWQ / SX)).astype(f8))  # [768, 512]
        wk_t = np.ascontiguousarray((wk_eff.T * (SWK / SX)).astype(f8))
    else:
        wq_t = np.ascontiguousarray(wq_eff.T.astype(bf16))   # [768, 512]
        wk_t = np.ascontiguousarray(wk_eff.T.astype(bf16))
    wv_t = np.ascontiguousarray((wv_eff.T / tscale).astype(bf16))
    wf_t = np.ascontiguousarray(wfin.T)                  # [512, 256] fp32
    bq_c = np.ascontiguousarray(bq_eff.reshape(HCH, P).T)  # [128, 4] fp32
    bk_c = np.ascontiguousarray(bk_eff.reshape(HCH, P).T)
    bv_r = np.ascontiguousarray(bv_eff.reshape(1, HIDDEN_DIM).astype(bf16))
    with_v_bias = bool(np.any(bv_eff != 0))
    with_qk_bias = bool(np.any(bq_eff != 0) or np.any(bk_eff != 0))

    ci = np.asarray(cell_idx).astype(np.int32)             # [2048, 64]
    ln = np.maximum(np.asarray(cell_len).astype(np.int64), 1)
    ln = np.minimum(ln, MAX_LEN).astype(np.int32)          # [2048]
    pos = np.arange(MAX_LEN, dtype=np.int32)
    valid = pos[None, :] < ln[:, None]                     # [2048, 64]
    maskb_full = np.where(valid, f32(0.0), f32(-1e30))     # [2048, 64]
    u_full = valid.astype(bf16)                            # exact 0/1 mask
    inv_len = (1.0 / ln.astype(f32))                       # host-side mean-pool

    in_maps = []
    for core in range(N_CORES):
        cs = slice(core * CELLS_PER_CORE, (core + 1) * CELLS_PER_CORE)
        T4 = TILES_PER_BLOCK
        idx_c = np.ascontiguousarray(
            ci[cs].reshape(BLOCKS, T4, P).transpose(0, 2, 1).reshape(BLOCKS * P, T4))
        mb_c = np.ascontiguousarray(
            maskb_full[cs].reshape(BLOCKS, T4, P).transpose(0, 2, 1)
            .reshape(BLOCKS * P, T4))
        u_c = u_full[cs]                                   # [256, 64]
        u2_c = np.zeros((TILES_PER_CORE, P, 2), bf16)
        u2_c[:, 0:64, 0] = u_c[0::2]
        u2_c[:, 64:128, 1] = u_c[1::2]
        u2_c = np.ascontiguousarray(
            u2_c.reshape(BLOCKS, T4, P, 2).transpose(0, 2, 1, 3)
            .reshape(BLOCKS * P, 2 * T4))
        in_maps.append({
            "table": cf,
            "wq_t": wq_t, "wk_t": wk_t, "wv_t": wv_t, "wf_t": wf_t,
            "bq_c": bq_c, "bk_c": bk_c, "bv_r": bv_r,
            "idx": idx_c, "maskb": mb_c,
            "u2": u2_c,
        })
    return in_maps, b_final, inv_len, (with_v_bias, with_qk_bias)


_NC_CACHE: dict = {}


def get_nc(flags):
    key = (flags, tuple(sorted(CFG.items())))
    if key not in _NC_CACHE:
        _NC_CACHE[key] = build_kernel(flags)
    return _NC_CACHE[key]


def kernel(**inputs) -> np.ndarray:
    in_maps, b_final, inv_len, flags = preprocess(**inputs)
    nc = get_nc(flags)
    res = run_bass_kernel_spmd(nc, in_maps, list(range(N_CORES)))
    out = np.concatenate([res.results[i]["out"] for i in range(N_CORES)], axis=0)
    return (out * inv_len[:, None] + b_final[None, :]).astype(np.float32)
